# revision 22
# baseline (speedup 1.0000x reference)
"""Trainium2 Bass kernel for a 2-layer GCN (CompressedGNN).

Computation (see reference):
    h1 = relu(A_hat @ (x @ W1) + b1)
    h2 = A_hat @ (h1 @ W2) + b2
    out = h2 @ Wc + bc
with A_hat = D^-1/2 (A + I) D^-1/2 built from edge_index (multi-edges kept).

Strategy (8 NeuronCores, SPMD, one program):
  - dst nodes grouped into 79 blocks of 128 (+1 dummy); blocks are
    rank-sorted by edge count and dealt 8-per-position so all cores get an
    identical loop structure with balanced work. Node rows are permuted to
    (core, position, offset) order; AllGather concatenation then makes the
    full feature table addressable by permuted row id.
  - per layer: local feature matmul (nodes-on-psum-partitions, weights as
    moving operand) -> bf16 AllGather of the 1280-row shard -> per block:
    dma_gather of the block's (padded) edge messages, 128 edges/chunk, and
    a PE matmul per chunk with S[edge, dst_local] = edge norm weight
    (normalization + self loops folded into S) accumulating into PSUM.
    Bias via a K=1 matmul of ones x bias; relu on ACT during PSUM->SBUF.
  - transposed activations needed as the next layer's stationary operand
    are produced with PE transposes (128x128).
  - classifier is a local matmul; host reassembles/unpermutes rows.

K-major layout convention: logical [512, M] operands live in SBUF/DRAM as
[128, 4*M] with contraction-chunk k occupying columns [k*M, (k+1)*M).
"""

import numpy as np
import ml_dtypes

import concourse.bacc as bacc
import concourse.mybir as mybir
import concourse.tile as tile
from concourse.masks import make_identity
from concourse.bass_utils import run_bass_kernel_spmd

N_NODES = 10000
D_IN = 512
D_HID = 512
D_OUT = 100
NCORES = 8
P = 128
KC = D_HID // P            # 4 contraction chunks
NPOS = 10                  # dst-block positions per core
NP_CORE = NPOS * P         # 1280 padded rows per core
NTOT = NCORES * NP_CORE    # 10240 rows in the gathered table
NBLK = (N_NODES + P - 1) // P   # 79 real blocks

bf16 = mybir.dt.bfloat16
f32 = mybir.dt.float32
i16 = mybir.dt.int16

_COMPILED = {}


def _kmajor(a):
    """[KC*P, M] -> [P, KC*M] with chunk k at columns [k*M, (k+1)*M)."""
    km, m = a.shape
    assert km == KC * P
    return np.concatenate([a[k * P:(k + 1) * P] for k in range(KC)], axis=1)


# ----------------------------------------------------------------------------
# host-side preprocessing
# ----------------------------------------------------------------------------

def _preprocess(x, edge_index, W1, b1, W2, b2, Wc, bc):
    src = np.asarray(edge_index[0], dtype=np.int64)
    dst = np.asarray(edge_index[1], dtype=np.int64)
    n = N_NODES

    deg = 1.0 + np.bincount(dst, minlength=n).astype(np.float64)
    dinv = 1.0 / np.sqrt(deg)
    w = (dinv[src] * dinv[dst]).astype(np.float32)

    # fold self loops into the edge list
    loops = np.arange(n, dtype=np.int64)
    s_all = np.concatenate([src, loops])
    d_all = np.concatenate([dst, loops])
    w_all = np.concatenate([w, (dinv * dinv).astype(np.float32)])

    blk = d_all // P
    cnt = np.bincount(blk, minlength=NBLK)
    c_b = np.maximum(1, -(-cnt // P))                    # chunks per block
    c_b80 = np.concatenate([c_b, [1]])                   # dummy block 79
    ranks = np.argsort(-c_b80, kind="stable")
    assign = ranks.reshape(NPOS, NCORES)                 # [pos, core] -> block
    C_B = [int(c_b80[assign[j]].max()) for j in range(NPOS)]
    chunk_off = np.concatenate([[0], np.cumsum(C_B)]).astype(int)
    total_chunks = int(chunk_off[-1])

    # permuted row id for every node
    permrow = np.zeros(n, dtype=np.int64)
    for j in range(NPOS):
        for k in range(NCORES):
            b = assign[j, k]
            if b >= NBLK:
                continue
            lo, hi = b * P, min((b + 1) * P, n)
            nodes = np.arange(lo, hi)
            permrow[nodes] = k * NP_CORE + j * P + (nodes - lo)

    # edges grouped by dst block
    eorder = np.argsort(blk, kind="stable")
    estart = np.concatenate([[0], np.cumsum(cnt)]).astype(int)

    src_perm = permrow[s_all]                            # gather row ids

    S_cores = []
    idx_cores = []
    xT_cores = []
    x32 = np.asarray(x, dtype=np.float32)
    for k in range(NCORES):
        S_k = np.zeros((P, total_chunks * P), dtype=ml_dtypes.bfloat16)
        idx_k = np.zeros((P, total_chunks * 8), dtype=np.int16)
        xT_k = np.zeros((D_IN, NP_CORE), dtype=np.float32)
        for j in range(NPOS):
            b = assign[j, k]
            cap = C_B[j] * P
            off = int(chunk_off[j])
            idxl = np.zeros(cap, dtype=np.int16)
            if b < NBLK and cnt[b] > 0:
                e = eorder[estart[b]:estart[b + 1]]
                ne = len(e)
                assert ne <= cap
                rows = src_perm[e].astype(np.int16)
                order = np.argsort(rows, kind="stable")
                e = e[order]
                idxl[:ne] = rows[order]
                dloc = (d_all[e] - b * P).astype(np.int64)
                S2 = np.zeros((cap, P), dtype=np.float32)
                S2[np.arange(ne), dloc] = w_all[e]
                S_k[:, off * P:(off + C_B[j]) * P] = (
                    S2.reshape(C_B[j], P, P).transpose(1, 0, 2)
                    .reshape(P, C_B[j] * P).astype(ml_dtypes.bfloat16))
            if b < NBLK:
                lo, hi = b * P, min((b + 1) * P, n)
                xT_k[:, j * P:j * P + (hi - lo)] = x32[lo:hi].T
            # column-major 16-wrap layout, replicated to 128 partitions
            idx_k[:, off * 8:(off + C_B[j]) * 8] = np.tile(
                idxl.reshape(-1, 16).T, (8, 1))
        S_cores.append(S_k)
        idx_cores.append(idx_k)
        xT_cores.append(_kmajor(xT_k).astype(ml_dtypes.bfloat16))

    weights = {
        "W1": _kmajor(np.asarray(W1, np.float32)).astype(ml_dtypes.bfloat16),
        "W2": _kmajor(np.asarray(W2, np.float32)).astype(ml_dtypes.bfloat16),
        "Wc": _kmajor(np.asarray(Wc, np.float32)).astype(ml_dtypes.bfloat16),
        "b1": np.asarray(b1, np.float32).astype(ml_dtypes.bfloat16)[None, :],
        "b2": np.asarray(b2, np.float32).astype(ml_dtypes.bfloat16)[None, :],
        "bc": np.asarray(bc, np.float32).astype(ml_dtypes.bfloat16)[None, :],
    }
    return {
        "C_B": tuple(C_B),
        "total_chunks": total_chunks,
        "chunk_off": chunk_off,
        "permrow": permrow,
        "S_cores": S_cores,
        "idx_cores": idx_cores,
        "xT_cores": xT_cores,
        "weights": weights,
    }


# ----------------------------------------------------------------------------
# device program
# ----------------------------------------------------------------------------

def _build(C_B, total_chunks, chunk_off, spmd=True, unroll=1,
           skip_gather=False, skip_cc=False, skip_smm=False,
           skip_trans=False):
    nc = bacc.Bacc("TRN2", target_bir_lowering=False, debug=False,
                   num_devices=NCORES if spmd else 1, num_swdge_queues=2)

    xT_d = nc.dram_tensor("xT", [P, KC * NP_CORE], bf16, kind="ExternalInput")
    W1_d = nc.dram_tensor("W1", [P, KC * D_HID], bf16, kind="ExternalInput")
    W2_d = nc.dram_tensor("W2", [P, KC * D_HID], bf16, kind="ExternalInput")
    Wc_d = nc.dram_tensor("Wc", [P, KC * D_OUT], bf16, kind="ExternalInput")
    b1_d = nc.dram_tensor("b1", [1, D_HID], bf16, kind="ExternalInput")
    b2_d = nc.dram_tensor("b2", [1, D_HID], bf16, kind="ExternalInput")
    bc_d = nc.dram_tensor("bc", [1, D_OUT], bf16, kind="ExternalInput")
    S_d = nc.dram_tensor("S", [P, total_chunks * P], bf16,
                         kind="ExternalInput")
    idx_d = nc.dram_tensor("idx", [P, total_chunks * 8], i16,
                           kind="ExternalInput")
    out_d = nc.dram_tensor("out", [NP_CORE, D_OUT], f32,
                           kind="ExternalOutput")

    with tile.TileContext(nc) as tc:
        with (
            tc.tile_pool(name="const", bufs=1) as cpool,
            tc.tile_pool(name="sbuf", bufs=3) as sb,
            tc.tile_pool(name="gath", bufs=1) as gp,
            tc.tile_pool(name="psum", bufs=2, space="PSUM") as ps,
            tc.tile_pool(name="dram", bufs=1, space="DRAM") as dr,
            tc.tile_pool(name="dram_sh", bufs=1, space="DRAM") as drs,
        ):
            # ---------------- constants / resident tensors ----------------
            def load_const(name, dram, shape, dtype):
                t = cpool.tile(shape, dtype, tag=name)
                nc.sync.dma_start(t[:], dram[:])
                return t

            W1_sb = load_const("w1", W1_d, [P, KC * D_HID], bf16)
            W2_sb = load_const("w2", W2_d, [P, KC * D_HID], bf16)
            Wc_sb = load_const("wc", Wc_d, [P, KC * D_OUT], bf16)
            b1_sb = load_const("b1", b1_d, [1, D_HID], bf16)
            b2_sb = load_const("b2", b2_d, [1, D_HID], bf16)
            bc_sb = load_const("bc", bc_d, [1, D_OUT], bf16)
            S_sb = load_const("S", S_d, [P, total_chunks * P], bf16)
            idx_sb = load_const("idx", idx_d, [P, total_chunks * 8], i16)
            xT_sb = load_const("xT", xT_d, [P, KC * NP_CORE], bf16)

            ones_sb = cpool.tile([1, P], bf16, tag="ones")
            nc.vector.memset(ones_sb[:], 1.0)
            ident = cpool.tile([P, P], bf16, tag="ident")
            make_identity(nc, ident[:])

            hT = cpool.tile([P, KC * NP_CORE], bf16, tag="hT")
            if skip_trans:
                nc.vector.memset(hT[:], 0.0)

            g_loc = [dr.tile([NP_CORE, D_HID], bf16, tag=f"gloc{i}",
                             name=f"gloc{i}") for i in range(2 * unroll)]
            g_full = [drs.tile([NTOT, D_HID], bf16,
                               addr_space="Shared" if spmd else "Local",
                               tag=f"gfull{i}", name=f"gfull{i}")
                      for i in range(2 * unroll)]

            def mm_tiles(lhsT_big, rhs_big, n_out, out_ext=None,
                         bias=None, store_dram=None):
                """out[m] = lhsT_big[:,k-major m].T @ rhs_big (+ ones.T bias).

                lhsT_big: [P, KC*NP_CORE] k-major; rhs_big: [P, KC*n_out].
                """
                for m in range(NPOS):
                    acc = ps.tile([P, n_out], f32, space="PSUM", tag="mm")
                    for k in range(KC):
                        nc.tensor.matmul(
                            out=acc[:],
                            lhsT=lhsT_big[:, k * NP_CORE + m * P:
                                          k * NP_CORE + (m + 1) * P],
                            rhs=rhs_big[:, k * n_out:(k + 1) * n_out],
                            start=(k == 0),
                            stop=(k == KC - 1 and bias is None),
                        )
                    if bias is not None:
                        nc.tensor.matmul(out=acc[:], lhsT=ones_sb[:],
                                         rhs=bias[:], start=False, stop=True)
                    if out_ext is not None:
                        o32 = sb.tile([P, n_out], f32, tag="o32")
                        nc.vector.tensor_copy(o32[:], acc[:])
                        nc.sync.dma_start(
                            out_ext[m * P:(m + 1) * P, :], o32[:])
                    if store_dram is not None:
                        gt = sb.tile([P, n_out], bf16, tag="gsb")
                        nc.scalar.activation(
                            gt[:], acc[:], mybir.ActivationFunctionType.Copy)
                        nc.sync.dma_start(
                            store_dram[m * P:(m + 1) * P, :], gt[:])

            GSUB = 5   # chunks per dma_gather (ring capacity: keep small)
            NMT = 6    # explicit rotated gather buffers
            mts = []
            for i in range(NMT):
                t = gp.tile([P, GSUB, D_HID], bf16, tag=f"mt{i}",
                            name=f"mt{i}")
                # padded/trimmed gather lanes leave stale bytes behind;
                # clear once so they are never NaN for the S=0 matmuls
                nc.vector.memset(t[:].rearrange("p c e -> p (c e)"), 0.0)
                mts.append(t)
            state = {"g": 0}

            def aggregate(layer, g_full_t, bias, relu, hT_out):
                """per position: gather messages, S-matmul, bias, act, T."""
                for j in range(NPOS):
                    cb = C_B[j]
                    off = int(chunk_off[j])
                    acc = ps.tile([P, D_HID], f32, space="PSUM", tag="agg", bufs=3)
                    for c0 in range(0, cb, GSUB):
                        g = min(GSUB, cb - c0)
                        gi = state["g"]; state["g"] += 1
                        mt = mts[gi % NMT]
                        if not skip_gather:
                            nc.gpsimd.dma_gather(
                                mt[:, :g, :], g_full_t[:],
                                idx_sb[:, (off + c0) * 8:(off + c0 + g) * 8],
                                g * P, g * P, D_HID,
                                queue_num=gi % 2,
                            )
                        if not skip_smm:
                            for c in range(g):
                                nc.tensor.matmul(
                                    out=acc[:],
                                    lhsT=S_sb[:, (off + c0 + c) * P:
                                              (off + c0 + c + 1) * P],
                                    rhs=mt[:, c, :],
                                    start=(c0 == 0 and c == 0), stop=False,
                                )
                    nc.tensor.matmul(out=acc[:], lhsT=ones_sb[:], rhs=bias[:],
                                     start=(skip_smm), stop=True)
                    h_sb = sb.tile([P, D_HID], bf16, tag=f"h{layer}")
                    nc.scalar.activation(
                        h_sb[:], acc[:],
                        mybir.ActivationFunctionType.Relu if relu
                        else mybir.ActivationFunctionType.Copy)
                    # transposes for the next matmul's stationary operand
                    if not skip_trans:
                        for k in range(KC):
                            tp = ps.tile([P, P], bf16, space="PSUM", tag="tr")
                            nc.tensor.transpose(
                                out=tp[:], in_=h_sb[:, k * P:(k + 1) * P],
                                identity=ident[:])
                            nc.vector.tensor_copy(
                                hT_out[:, k * NP_CORE + j * P:
                                       k * NP_CORE + (j + 1) * P],
                                tp[:])

            def allgather(i):
                if skip_cc:
                    return
                if not spmd:
                    # single-core timing/sim variant: stand in for the
                    # collective with equivalent local DRAM traffic
                    for r in range(NCORES):
                        nc.sync.dma_start(
                            g_full[i][r * NP_CORE:(r + 1) * NP_CORE, :],
                            g_loc[i][:])
                    return
                nc.gpsimd.collective_compute(
                    "AllGather",
                    mybir.AluOpType.bypass,
                    replica_groups=[list(range(NCORES))],
                    ins=[g_loc[i].opt()],
                    outs=[g_full[i].opt()],
                )

            for _rep in range(unroll):
                # ---------------- layer 1 ----------------
                mm_tiles(xT_sb, W1_sb, D_HID, store_dram=g_loc[2 * _rep])
                allgather(2 * _rep)
                aggregate(1, g_full[2 * _rep], b1_sb, True, hT)

                # ---------------- layer 2 ----------------
                mm_tiles(hT, W2_sb, D_HID, store_dram=g_loc[2 * _rep + 1])
                allgather(2 * _rep + 1)
                aggregate(2, g_full[2 * _rep + 1], b2_sb, False, hT)

                # ---------------- classifier ----------------
                mm_tiles(hT, Wc_sb, D_OUT, out_ext=out_d, bias=bc_sb)

    nc.compile()
    return nc


def _get_program(C_B, total_chunks, chunk_off):
    key = tuple(C_B)
    if key not in _COMPILED:
        _COMPILED[key] = _build(C_B, total_chunks, chunk_off)
    return _COMPILED[key]


# ----------------------------------------------------------------------------
# entry point
# ----------------------------------------------------------------------------

def kernel(x, edge_index, W1, b1, W2, b2, Wc, bc, _want_trace=False,
           **trace_kwargs):
    pre = _preprocess(x, edge_index, W1, b1, W2, b2, Wc, bc)
    nc = _get_program(pre["C_B"], pre["total_chunks"], pre["chunk_off"])

    wts = pre["weights"]
    in_maps = []
    for k in range(NCORES):
        in_maps.append({
            "xT": pre["xT_cores"][k],
            "W1": wts["W1"], "W2": wts["W2"], "Wc": wts["Wc"],
            "b1": wts["b1"], "b2": wts["b2"], "bc": wts["bc"],
            "S": pre["S_cores"][k],
            "idx": pre["idx_cores"][k],
        })

    res = run_bass_kernel_spmd(nc, in_maps, core_ids=list(range(NCORES)),
                               trace=_want_trace, **trace_kwargs)

    big = np.concatenate([res.results[k]["out"] for k in range(NCORES)],
                         axis=0)
    out = big[pre["permrow"]].astype(np.float32)
    if _want_trace:
        return out, res
    return out
